# revision 7
# baseline (speedup 1.0000x reference)
"""Bass/TRN2 kernel for nn_BasicAttn: full softmax attention returning (context, attn).

Sharding: batch*heads (4*8=32 pairs) split across 8 NeuronCores, 4 pairs/core.
Per (b,h) pair on-core (S=2048, DK=64, scale=1/8):
  - Q,K,V loaded rounded to float32r; Q^T/K^T built via PE transposes.
  - S^T tiles [128k x 512q] = K@Q^T (f32r matmul), exp on ACT -> E^T (f32r).
  - PV: lhsT = [V | 1] so psum row 64 accumulates softmax denominators while
    rows 0..63 accumulate context^T.
  - attn [q,k] tiles: first KS columns recomputed as S=Q@K^T + exp, normalized
    in-place; remaining columns produced by PE-transposing E^T tiles with the
    1/denom scaling fused into the PSUM->SBUF copy.
  - context^T transposed back on PE, scaled by 1/denom, written per pair.
"""

import numpy as np

B, H, S_FULL, DK = 4, 8, 2048, 64
N_CORES = 8
PAIRS_PER_CORE = (B * H) // N_CORES
P = 128
CW = 512  # q-chunk width (fp32 moving-operand max)


def build_bass(S=S_FULL, PAIRS=PAIRS_PER_CORE, KS=512):
    """Build the per-core SPMD Bass program. KS = attn columns produced via
    re-matmul+exp (rest via PE transpose of E^T); must be a multiple of 512."""
    import concourse.bacc as bacc
    import concourse.tile as tile
    import concourse.mybir as mybir

    f32 = mybir.dt.float32
    f32r = mybir.dt.float32r
    AF = mybir.ActivationFunctionType

    NT = S // P      # 128-row tiles per pair
    NCH = S // CW    # 512-wide q chunks per pair
    KTS = KS // P    # k-tiles covered by the re-matmul path
    SCALE = 1.0 / np.sqrt(DK).astype(np.float32)

    nc = bacc.Bacc("TRN2", target_bir_lowering=False, debug=False)

    q_in = nc.dram_tensor("q_in", (PAIRS, S, DK), f32, kind="ExternalInput").ap()
    k_in = nc.dram_tensor("k_in", (PAIRS, S, DK), f32, kind="ExternalInput").ap()
    v_in = nc.dram_tensor("v_in", (PAIRS, S, DK), f32, kind="ExternalInput").ap()
    attn_out = nc.dram_tensor("attn_out", (PAIRS, S, S), f32, kind="ExternalOutput").ap()
    ctx_out = nc.dram_tensor("ctx_out", (PAIRS, S, DK), f32, kind="ExternalOutput").ap()

    with tile.TileContext(nc) as tc:
        with tc.tile_pool(name="singles", bufs=1) as singles, \
             tc.tile_pool(name="loads", bufs=2) as loads, \
             tc.tile_pool(name="pairb", bufs=2) as pairb, \
             tc.tile_pool(name="et", bufs=2) as etp, \
             tc.tile_pool(name="attn", bufs=3) as attnp, \
             tc.tile_pool(name="small", bufs=4) as small, \
             tc.tile_pool(name="mmps", bufs=2, space="PSUM") as mmps, \
             tc.tile_pool(name="ctxps", bufs=2, space="PSUM") as ctxps, \
             tc.tile_pool(name="trps", bufs=2, space="PSUM") as trps:

            ident_dram = nc.inline_tensor(np.eye(P, dtype=np.float32), name="ident")
            aug_np = np.zeros((P, NT, 32), dtype=np.float32)
            aug_np[:, :, 0] = 1.0
            aug_dram = nc.inline_tensor(aug_np, name="augcols")
            ident = singles.tile([P, P], f32r)
            nc.gpsimd.dma_start(out=ident, in_=ident_dram.ap())

            for pr in range(PAIRS):
                # ---- stage A: load + build Q^T, K^T (f32r), V|1 ----
                # Q: contiguous per-partition load; q index = p*NT + t
                q_ld = loads.tile([P, NT, DK], f32r, tag="qld")
                nc.gpsimd.dma_start(
                    out=q_ld, in_=q_in[pr].rearrange("(p t) d -> p t d", t=NT)
                )
                # K, V: tiled load; k index = t*P + p (natural within-tile order)
                k_ld = loads.tile([P, NT, DK], f32r, tag="kld")
                nc.gpsimd.dma_start(
                    out=k_ld, in_=k_in[pr].rearrange("(t p) d -> p t d", p=P)
                )
                v_aug = pairb.tile([P, NT, DK + 32], f32r, tag="vaug")
                nc.gpsimd.dma_start(
                    out=v_aug[:, :, :DK], in_=v_in[pr].rearrange("(t p) d -> p t d", p=P)
                )
                nc.gpsimd.dma_start(out=v_aug[:, :, DK:], in_=aug_dram.ap())

                qt = pairb.tile([DK, S], f32r, tag="qt")
                kt_b = pairb.tile([DK, S], f32r, tag="kt")
                for t4 in range(0, NT, 4):
                    psq = trps.tile([P, 4, P], f32r, tag="tr")
                    for j in range(4):
                        nc.tensor.transpose(
                            psq[:DK, j, :], q_ld[:, t4 + j, :], ident
                        )
                    nc.any.tensor_copy(
                        out=qt[:, t4 * P : (t4 + 4) * P].rearrange(
                            "d (j c) -> d j c", j=4
                        ),
                        in_=psq[:DK],
                    )
                    psk = trps.tile([P, 4, P], f32r, tag="tr")
                    for j in range(4):
                        nc.tensor.transpose(
                            psk[:DK, j, :], k_ld[:, t4 + j, :], ident
                        )
                    nc.any.tensor_copy(
                        out=kt_b[:, t4 * P : (t4 + 4) * P].rearrange(
                            "d (j c) -> d j c", j=4
                        ),
                        in_=psk[:DK],
                    )

                ctx_sb = pairb.tile([P, NT, DK], f32, tag="ctxsb")

                for qc in range(NCH):
                    # ---- stage B: E^T tiles + PV/denominator accumulation ----
                    et = etp.tile([P, NT, CW], f32r, tag="et")
                    ctxT = ctxps.tile([DK + 32, CW], f32, tag="ctxT")
                    for kt2 in range(0, NT, 2):
                        st = mmps.tile([P, 2, CW], f32, tag="mm")
                        for j in range(2):
                            kt = kt2 + j
                            nc.tensor.matmul(
                                st[:, j, :],
                                lhsT=kt_b[:, kt * P : (kt + 1) * P],
                                rhs=qt[:, qc * CW : (qc + 1) * CW],
                                start=True,
                                stop=True,
                            )
                        nc.scalar.activation(
                            out=et[:, kt2 : kt2 + 2, :],
                            in_=st,
                            func=AF.Exp,
                            scale=float(SCALE),
                        )
                        for j in range(2):
                            kt = kt2 + j
                            nc.tensor.matmul(
                                ctxT,
                                lhsT=v_aug[:, kt, :],
                                rhs=et[:, kt, :],
                                start=(kt == 0),
                                stop=(kt == NT - 1),
                            )

    # context^T rows 0..63 plus reciprocal-denominator row 64, transposed
                    # together so the recips land per-partition for q.
                    stats_sb = small.tile([DK + 32, CW], f32r, tag="ctxTsb")
                    nc.any.tensor_copy(out=stats_sb, in_=ctxT)
                    with nc.allow_low_precision(
                        reason="f32r reciprocal: 6e-5 rel rounding is within budget"
                    ):
                        nc.vector.reciprocal(
                            out=stats_sb[DK : DK + 1, :], in_=ctxT[DK : DK + 1, :]
                        )
                    recip_t = small.tile([P, 4, 1], f32, tag="recipt")
                    psc = trps.tile([P, 4, P], f32r, tag="tr")
                    for g in range(4):
                        nc.tensor.transpose(
                            psc[:, g, : DK + 32],
                            stats_sb[:, g * P : (g + 1) * P],
                            ident[: DK + 32, : DK + 32],
                        )
                        nc.any.tensor_copy(
                            out=recip_t[:, g, :], in_=psc[:, g, DK : DK + 1].bitcast(f32)
                        )
                        nc.any.tensor_scalar_mul(
                            ctx_sb[:, qc * 4 + g, :],
                            psc[:, g, :DK].bitcast(f32),
                            recip_t[:, g, :],
                        )

                    # ---- stage C: attn tiles for this chunk ----
                    for g in range(4):
                        gt = qc * 4 + g
                        attn_t = attnp.tile([P, S], f32, tag="attn")
                        # re-matmul path: columns [0, KS)
                        for c in range(KTS // 4):
                            sps = mmps.tile([P, 2, CW], f32, tag="mm")
                            for j in range(2):
                                nc.tensor.matmul(
                                    sps[:, j, :],
                                    lhsT=qt[:, gt * P : (gt + 1) * P],
                                    rhs=kt_b[:, (c * 2 + j) * CW : (c * 2 + j + 1) * CW],
                                    start=True,
                                    stop=True,
                                )
                            nc.scalar.activation(
                                out=attn_t[:, c * 2 * CW : (c * 2 + 2) * CW].rearrange(
                                    "q (j c) -> q j c", j=2
                                ),
                                in_=sps,
                                func=AF.Exp,
                                scale=float(SCALE),
                            )
                        if KTS:
                            nc.any.tensor_scalar_mul(
                                attn_t[:, :KS], attn_t[:, :KS], recip_t[:, g, :]
                            )
                        # transpose path: columns [KS, S)
                        for kt4 in range(KTS, NT, 4):
                            tps = trps.tile([P, 4, P], f32r, tag="tr")
                            for j in range(4):
                                nc.tensor.transpose(
                                    tps[:, j, :],
                                    et[:, kt4 + j, g * P : (g + 1) * P],
                                    ident,
                                )
                            nc.any.tensor_scalar_mul(
                                attn_t[:, kt4 * P : (kt4 + 4) * P].rearrange(
                                    "q (j c) -> q j c", j=4
                                ),
                                tps.bitcast(f32),
                                recip_t[:, g, :],
                            )
                        nc.sync.dma_start(
                            out=attn_out[pr].rearrange("(p t) k -> p t k", t=NT)[
                                :, gt, :
                            ],
                            in_=attn_t,
                        )

                nc.sync.dma_start(
                    out=ctx_out[pr].rearrange("(p t) d -> p t d", t=NT),
                    in_=ctx_sb,
                )

    nc.compile()
    return nc


_CACHE = {}


def _get_bass(**kw):
    key = tuple(sorted(kw.items()))
    if key not in _CACHE:
        _CACHE[key] = build_bass(**kw)
    return _CACHE[key]


def kernel(Q, K, V, attn_mask=None):
    """Full-input entry point: Q,K,V [4,8,2048,64] fp32 -> (context, attn)."""
    from concourse.bass_utils import run_bass_kernel_spmd

    Bq, Hq, S, D = Q.shape
    n_pairs = Bq * Hq
    QQ = np.ascontiguousarray(np.asarray(Q, dtype=np.float32).reshape(n_pairs, S, D))
    KK = np.ascontiguousarray(np.asarray(K, dtype=np.float32).reshape(n_pairs, S, D))
    VV = np.ascontiguousarray(np.asarray(V, dtype=np.float32).reshape(n_pairs, S, D))

    nc = _get_bass()
    per = n_pairs // N_CORES
    in_maps = [
        {
            "q_in": QQ[c * per : (c + 1) * per],
            "k_in": KK[c * per : (c + 1) * per],
            "v_in": VV[c * per : (c + 1) * per],
        }
        for c in range(N_CORES)
    ]
    res = run_bass_kernel_spmd(nc, in_maps, core_ids=list(range(N_CORES)))
    attn = np.concatenate([r["attn_out"] for r in res.results], axis=0).reshape(
        Bq, Hq, S, S
    )
    ctx = np.concatenate([r["ctx_out"] for r in res.results], axis=0).reshape(
        Bq, Hq, S, D
    )
    return ctx, attn


# revision 10
# speedup vs baseline: 2.7515x; 2.7515x over previous
"""Bass/TRN2 kernel for nn_BasicAttn: full softmax attention returning (context, attn).

Sharding: batch*heads (4*8=32 pairs) split across 8 NeuronCores, 4 pairs/core.
Per (b,h) pair on-core (S=2048, DK=64, scale=1/8):
  - Q,K,V loaded rounded to float32r; Q^T/K^T built via PE transposes.
  - S^T tiles [128k x 512q] = K@Q^T (f32r matmul), exp on ACT -> E^T (f32r).
  - PV: lhsT = [V | 1] so psum row 64 accumulates softmax denominators while
    rows 0..63 accumulate context^T.
  - attn [q,k] tiles: first KS columns recomputed as S=Q@K^T + exp, normalized
    in-place; remaining columns produced by PE-transposing E^T tiles with the
    1/denom scaling fused into the PSUM->SBUF copy.
  - context^T transposed back on PE, scaled by 1/denom, written per pair.
"""

import numpy as np

B, H, S_FULL, DK = 4, 8, 2048, 64
N_CORES = 8
PAIRS_PER_CORE = (B * H) // N_CORES
P = 128
CW = 512  # q-chunk width (fp32 moving-operand max)


def build_bass(S=S_FULL, PAIRS=PAIRS_PER_CORE, KS=512, repeat=1):
    """Build the per-core SPMD Bass program. KS = attn columns produced via
    re-matmul+exp (rest via PE transpose of E^T); must be a multiple of 512.
    repeat>1 wraps the whole computation in a hardware loop (benchmarking)."""
    import contextlib
    import concourse.bacc as bacc
    import concourse.tile as tile
    import concourse.mybir as mybir

    f32 = mybir.dt.float32
    f32r = mybir.dt.float32r
    AF = mybir.ActivationFunctionType

    NT = S // P      # 128-row tiles per pair
    NCH = S // CW    # 512-wide q chunks per pair
    KTS = KS // P    # k-tiles covered by the re-matmul path
    SCALE = 1.0 / np.sqrt(DK).astype(np.float32)

    nc = bacc.Bacc("TRN2", target_bir_lowering=False, debug=False)

    q_in = nc.dram_tensor("q_in", (PAIRS, S, DK), f32, kind="ExternalInput").ap()
    k_in = nc.dram_tensor("k_in", (PAIRS, S, DK), f32, kind="ExternalInput").ap()
    v_in = nc.dram_tensor("v_in", (PAIRS, S, DK), f32, kind="ExternalInput").ap()
    attn_out = nc.dram_tensor("attn_out", (PAIRS, S, S), f32, kind="ExternalOutput").ap()
    ctx_out = nc.dram_tensor("ctx_out", (PAIRS, S, DK), f32, kind="ExternalOutput").ap()

    with tile.TileContext(nc) as tc:
        with tc.tile_pool(name="singles", bufs=1) as singles, \
             tc.tile_pool(name="loads", bufs=2) as loads, \
             tc.tile_pool(name="pairb", bufs=2) as pairb, \
             tc.tile_pool(name="et", bufs=2) as etp, \
             tc.tile_pool(name="attn", bufs=3) as attnp, \
             tc.tile_pool(name="small", bufs=4) as small, \
             tc.tile_pool(name="mmps", bufs=2, space="PSUM") as mmps, \
             tc.tile_pool(name="ctxps", bufs=2, space="PSUM") as ctxps, \
             tc.tile_pool(name="trps", bufs=2, space="PSUM") as trps:

            ident_dram = nc.inline_tensor(np.eye(P, dtype=np.float32), name="ident")
            aug_np = np.zeros((P, NT, 32), dtype=np.float32)
            aug_np[:, :, 0] = 1.0
            aug_dram = nc.inline_tensor(aug_np, name="augcols")
            ident = singles.tile([P, P], f32r)
            nc.gpsimd.dma_start(out=ident, in_=ident_dram.ap())

            rep_ctx = tc.For_i(0, repeat, 1) if repeat > 1 else contextlib.nullcontext()
            with rep_ctx:
              for pr in range(PAIRS):
                # ---- stage A: load + build Q^T, K^T (f32r), V|1 ----
                # Q: contiguous per-partition load; q index = p*NT + t
                q_ld = loads.tile([P, NT, DK], f32r, tag="qld")
                nc.gpsimd.dma_start(
                    out=q_ld, in_=q_in[pr].rearrange("(p t) d -> p t d", t=NT)
                )
                # K, V: tiled load; k index = t*P + p (natural within-tile order)
                k_ld = loads.tile([P, NT, DK], f32r, tag="kld")
                nc.gpsimd.dma_start(
                    out=k_ld, in_=k_in[pr].rearrange("(t p) d -> p t d", p=P)
                )
                v_aug = pairb.tile([P, NT, DK + 32], f32r, tag="vaug")
                nc.gpsimd.dma_start(
                    out=v_aug[:, :, :DK], in_=v_in[pr].rearrange("(t p) d -> p t d", p=P)
                )
                nc.gpsimd.dma_start(out=v_aug[:, :, DK:], in_=aug_dram.ap())

                qt = pairb.tile([DK, S], f32r, tag="qt")
                kt_b = pairb.tile([DK, S], f32r, tag="kt")
                for t4 in range(0, NT, 4):
                    psq = trps.tile([P, 4, P], f32r, tag="tr")
                    for j in range(4):
                        nc.tensor.transpose(
                            psq[:DK, j, :], q_ld[:, t4 + j, :], ident
                        )
                    nc.any.tensor_copy(
                        out=qt[:, t4 * P : (t4 + 4) * P].rearrange(
                            "d (j c) -> d j c", j=4
                        ),
                        in_=psq[:DK],
                    )
                    psk = trps.tile([P, 4, P], f32r, tag="tr")
                    for j in range(4):
                        nc.tensor.transpose(
                            psk[:DK, j, :], k_ld[:, t4 + j, :], ident
                        )
                    nc.any.tensor_copy(
                        out=kt_b[:, t4 * P : (t4 + 4) * P].rearrange(
                            "d (j c) -> d j c", j=4
                        ),
                        in_=psk[:DK],
                    )

                ctx_sb = pairb.tile([P, NT, DK], f32, tag="ctxsb")

                for qc in range(NCH):
                    # ---- stage B: E^T tiles + PV/denominator accumulation ----
                    et = etp.tile([P, NT, CW], f32r, tag="et")
                    ctxT = ctxps.tile([DK + 32, CW], f32, tag="ctxT")
                    for kt2 in range(0, NT, 2):
                        st = mmps.tile([P, 2, CW], f32, tag="mm")
                        for j in range(2):
                            kt = kt2 + j
                            nc.tensor.matmul(
                                st[:, j, :],
                                lhsT=kt_b[:, kt * P : (kt + 1) * P],
                                rhs=qt[:, qc * CW : (qc + 1) * CW],
                                start=True,
                                stop=True,
                            )
                        nc.scalar.activation(
                            out=et[:, kt2 : kt2 + 2, :],
                            in_=st,
                            func=AF.Exp,
                            scale=float(SCALE),
                        )
                        for j in range(2):
                            kt = kt2 + j
                            nc.tensor.matmul(
                                ctxT,
                                lhsT=v_aug[:, kt, :],
                                rhs=et[:, kt, :],
                                start=(kt == 0),
                                stop=(kt == NT - 1),
                            )

    # context^T rows 0..63 plus reciprocal-denominator row 64, transposed
                    # together so the recips land per-partition for q.
                    stats_sb = small.tile([DK + 32, CW], f32r, tag="ctxTsb")
                    nc.any.tensor_copy(out=stats_sb, in_=ctxT)
                    with nc.allow_low_precision(
                        reason="f32r reciprocal: 6e-5 rel rounding is within budget"
                    ):
                        nc.vector.reciprocal(
                            out=stats_sb[DK : DK + 1, :], in_=ctxT[DK : DK + 1, :]
                        )
                    recip_t = small.tile([P, 4, 1], f32, tag="recipt")
                    psc = trps.tile([P, 4, P], f32r, tag="tr")
                    for g in range(4):
                        nc.tensor.transpose(
                            psc[:, g, : DK + 32],
                            stats_sb[:, g * P : (g + 1) * P],
                            ident[: DK + 32, : DK + 32],
                        )
                        nc.any.tensor_copy(
                            out=recip_t[:, g, :], in_=psc[:, g, DK : DK + 1].bitcast(f32)
                        )
                        nc.any.tensor_scalar_mul(
                            ctx_sb[:, qc * 4 + g, :],
                            psc[:, g, :DK].bitcast(f32),
                            recip_t[:, g, :],
                        )

                    # ---- stage C: attn tiles for this chunk ----
                    for g in range(4):
                        gt = qc * 4 + g
                        attn_t = attnp.tile([P, S], f32, tag="attn")
                        # re-matmul path: columns [0, KS)
                        c = 0
                        while c < KS // CW:
                            take = 2 if c + 1 < KS // CW else 1
                            sps = mmps.tile([P, 2, CW], f32, tag="mm")
                            for j in range(take):
                                nc.tensor.matmul(
                                    sps[:, j, :],
                                    lhsT=qt[:, gt * P : (gt + 1) * P],
                                    rhs=kt_b[:, (c + j) * CW : (c + j + 1) * CW],
                                    start=True,
                                    stop=True,
                                )
                            nc.scalar.activation(
                                out=attn_t[:, c * CW : (c + take) * CW].rearrange(
                                    "q (j c) -> q j c", j=take
                                ),
                                in_=sps[:, :take, :],
                                func=AF.Exp,
                                scale=float(SCALE),
                            )
                            c += take
                        if KTS:
                            nc.any.tensor_scalar_mul(
                                attn_t[:, :KS], attn_t[:, :KS], recip_t[:, g, :]
                            )
                        # transpose path: columns [KS, S)
                        for kt4 in range(KTS, NT, 4):
                            tps = trps.tile([P, 4, P], f32r, tag="tr")
                            for j in range(4):
                                nc.tensor.transpose(
                                    tps[:, j, :],
                                    et[:, kt4 + j, g * P : (g + 1) * P],
                                    ident,
                                )
                            nc.any.tensor_scalar_mul(
                                attn_t[:, kt4 * P : (kt4 + 4) * P].rearrange(
                                    "q (j c) -> q j c", j=4
                                ),
                                tps.bitcast(f32),
                                recip_t[:, g, :],
                            )
                        nc.sync.dma_start(
                            out=attn_out[pr].rearrange("(p t) k -> p t k", t=NT)[
                                :, gt, :
                            ],
                            in_=attn_t,
                        )

                nc.sync.dma_start(
                    out=ctx_out[pr].rearrange("(p t) d -> p t d", t=NT),
                    in_=ctx_sb,
                )

    nc.compile()
    return nc


_CACHE = {}


def _get_bass(**kw):
    key = tuple(sorted(kw.items()))
    if key not in _CACHE:
        _CACHE[key] = build_bass(**kw)
    return _CACHE[key]


def kernel(Q, K, V, attn_mask=None):
    """Full-input entry point: Q,K,V [4,8,2048,64] fp32 -> (context, attn)."""
    from concourse.bass_utils import run_bass_kernel_spmd

    Bq, Hq, S, D = Q.shape
    n_pairs = Bq * Hq
    QQ = np.ascontiguousarray(np.asarray(Q, dtype=np.float32).reshape(n_pairs, S, D))
    KK = np.ascontiguousarray(np.asarray(K, dtype=np.float32).reshape(n_pairs, S, D))
    VV = np.ascontiguousarray(np.asarray(V, dtype=np.float32).reshape(n_pairs, S, D))

    nc = _get_bass()
    per = n_pairs // N_CORES
    in_maps = [
        {
            "q_in": QQ[c * per : (c + 1) * per],
            "k_in": KK[c * per : (c + 1) * per],
            "v_in": VV[c * per : (c + 1) * per],
        }
        for c in range(N_CORES)
    ]
    res = run_bass_kernel_spmd(nc, in_maps, core_ids=list(range(N_CORES)))
    attn = np.concatenate([r["attn_out"] for r in res.results], axis=0).reshape(
        Bq, Hq, S, S
    )
    ctx = np.concatenate([r["ctx_out"] for r in res.results], axis=0).reshape(
        Bq, Hq, S, D
    )
    return ctx, attn


# revision 12
# speedup vs baseline: 3.0663x; 1.1144x over previous
"""Bass/TRN2 kernel for nn_BasicAttn: full softmax attention returning (context, attn).

Sharding: batch*heads (4*8=32 pairs) split across 8 NeuronCores, 4 pairs/core.
Per (b,h) pair on-core (S=2048, DK=64, scale=1/8):
  - Q,K,V loaded rounded to float32r; Q^T/K^T built via PE transposes.
  - S^T tiles [128k x 512q] = K@Q^T (f32r matmul), exp on ACT -> E^T (f32r).
  - PV: lhsT = [V | 1] so psum row 64 accumulates softmax denominators while
    rows 0..63 accumulate context^T.
  - attn [q,k] tiles: first KS columns recomputed as S=Q@K^T + exp, normalized
    in-place; remaining columns produced by PE-transposing E^T tiles with the
    1/denom scaling fused into the PSUM->SBUF copy.
  - context^T transposed back on PE, scaled by 1/denom, written per pair.
"""

import numpy as np

B, H, S_FULL, DK = 4, 8, 2048, 64
N_CORES = 8
PAIRS_PER_CORE = (B * H) // N_CORES
P = 128
CW = 512  # q-chunk width (fp32 moving-operand max)


def build_bass(S=S_FULL, PAIRS=PAIRS_PER_CORE, KS=512, repeat=1):
    """Build the per-core SPMD Bass program. KS = attn columns produced via
    re-matmul+exp (rest via PE transpose of E^T); must be a multiple of 512.
    repeat>1 wraps the whole computation in a hardware loop (benchmarking)."""
    import contextlib
    import concourse.bacc as bacc
    import concourse.tile as tile
    import concourse.mybir as mybir

    f32 = mybir.dt.float32
    f32r = mybir.dt.float32r
    AF = mybir.ActivationFunctionType

    NT = S // P      # 128-row tiles per pair
    NCH = S // CW    # 512-wide q chunks per pair
    KTS = KS // P    # k-tiles covered by the re-matmul path
    SCALE = 1.0 / np.sqrt(DK).astype(np.float32)

    nc = bacc.Bacc("TRN2", target_bir_lowering=False, debug=False)

    q_in = nc.dram_tensor("q_in", (PAIRS, S, DK), f32, kind="ExternalInput").ap()
    k_in = nc.dram_tensor("k_in", (PAIRS, S, DK), f32, kind="ExternalInput").ap()
    v_in = nc.dram_tensor("v_in", (PAIRS, S, DK), f32, kind="ExternalInput").ap()
    attn_out = nc.dram_tensor("attn_out", (PAIRS, S, S), f32, kind="ExternalOutput").ap()
    ctx_out = nc.dram_tensor("ctx_out", (PAIRS, S, DK), f32, kind="ExternalOutput").ap()

    with tile.TileContext(nc) as tc:
        with tc.tile_pool(name="singles", bufs=1) as singles, \
             tc.tile_pool(name="loads", bufs=2) as loads, \
             tc.tile_pool(name="pairb", bufs=2) as pairb, \
             tc.tile_pool(name="et", bufs=2) as etp, \
             tc.tile_pool(name="attn", bufs=2) as attnp, \
             tc.tile_pool(name="small", bufs=4) as small, \
             tc.tile_pool(name="mmps", bufs=2, space="PSUM") as mmps, \
             tc.tile_pool(name="ctxps", bufs=2, space="PSUM") as ctxps, \
             tc.tile_pool(name="trps", bufs=2, space="PSUM") as trps:

            ident_dram = nc.inline_tensor(np.eye(P, dtype=np.float32), name="ident")
            aug_np = np.zeros((P, NT, 32), dtype=np.float32)
            aug_np[:, :, 0] = 1.0
            aug_dram = nc.inline_tensor(aug_np, name="augcols")
            ident = singles.tile([P, P], f32r)
            nc.gpsimd.dma_start(out=ident, in_=ident_dram.ap())

            rep_ctx = tc.For_i(0, repeat, 1) if repeat > 1 else contextlib.nullcontext()
            with rep_ctx:
              for pr in range(PAIRS):
                # ---- stage A: load + build Q^T, K^T (f32r), V|1 ----
                # Q: contiguous per-partition load; q index = p*NT + t
                q_ld = loads.tile([P, NT, DK], f32r, tag="qld")
                nc.gpsimd.dma_start(
                    out=q_ld, in_=q_in[pr].rearrange("(p t) d -> p t d", t=NT)
                )
                # K, V: tiled load; k index = t*P + p (natural within-tile order)
                k_ld = loads.tile([P, NT, DK], f32r, tag="kld")
                nc.gpsimd.dma_start(
                    out=k_ld, in_=k_in[pr].rearrange("(t p) d -> p t d", p=P)
                )
                v_aug = pairb.tile([P, NT, DK + 32], f32r, tag="vaug")
                nc.gpsimd.dma_start(
                    out=v_aug[:, :, :DK], in_=v_in[pr].rearrange("(t p) d -> p t d", p=P)
                )
                nc.gpsimd.dma_start(out=v_aug[:, :, DK:], in_=aug_dram.ap())

                qt = pairb.tile([DK, S], f32r, tag="qt")
                kt_b = pairb.tile([DK, S], f32r, tag="kt")
                for t4 in range(0, NT, 4):
                    psq = trps.tile([P, 4, P], f32r, tag="tr")
                    for j in range(4):
                        nc.tensor.transpose(
                            psq[:DK, j, :], q_ld[:, t4 + j, :], ident
                        )
                    nc.vector.tensor_copy(
                        out=qt[:, t4 * P : (t4 + 4) * P].rearrange(
                            "d (j c) -> d j c", j=4
                        ),
                        in_=psq[:DK],
                    )
                    psk = trps.tile([P, 4, P], f32r, tag="tr")
                    for j in range(4):
                        nc.tensor.transpose(
                            psk[:DK, j, :], k_ld[:, t4 + j, :], ident
                        )
                    nc.vector.tensor_copy(
                        out=kt_b[:, t4 * P : (t4 + 4) * P].rearrange(
                            "d (j c) -> d j c", j=4
                        ),
                        in_=psk[:DK],
                    )

                ctx_sb = pairb.tile([P, NT, DK], f32, tag="ctxsb")

                for qc in range(NCH):
                    # ---- stage B: E^T tiles + PV/denominator accumulation ----
                    et = etp.tile([P, NT, CW], f32r, tag="et")
                    ctxT = ctxps.tile([DK + 32, CW], f32, tag="ctxT")
                    for kt2 in range(0, NT, 2):
                        st = mmps.tile([P, 2, CW], f32, tag="mm")
                        for j in range(2):
                            kt = kt2 + j
                            nc.tensor.matmul(
                                st[:, j, :],
                                lhsT=kt_b[:, kt * P : (kt + 1) * P],
                                rhs=qt[:, qc * CW : (qc + 1) * CW],
                                start=True,
                                stop=True,
                            )
                        nc.scalar.activation(
                            out=et[:, kt2 : kt2 + 2, :],
                            in_=st,
                            func=AF.Exp,
                            scale=float(SCALE),
                        )
                        for j in range(2):
                            kt = kt2 + j
                            nc.tensor.matmul(
                                ctxT,
                                lhsT=v_aug[:, kt, :],
                                rhs=et[:, kt, :],
                                start=(kt == 0),
                                stop=(kt == NT - 1),
                            )

    # context^T rows 0..63 plus reciprocal-denominator row 64, transposed
                    # together so the recips land per-partition for q.
                    stats_sb = small.tile([DK + 32, CW], f32r, tag="ctxTsb")
                    nc.vector.tensor_copy(out=stats_sb, in_=ctxT)
                    with nc.allow_low_precision(
                        reason="f32r reciprocal: 6e-5 rel rounding is within budget"
                    ):
                        nc.vector.reciprocal(
                            out=stats_sb[DK : DK + 1, :], in_=ctxT[DK : DK + 1, :]
                        )
                    recip_t = small.tile([P, 4, 1], f32, tag="recipt")
                    psc = trps.tile([P, 4, P], f32r, tag="tr")
                    for g in range(4):
                        nc.tensor.transpose(
                            psc[:, g, : DK + 32],
                            stats_sb[:, g * P : (g + 1) * P],
                            ident[: DK + 32, : DK + 32],
                        )
                        nc.vector.tensor_copy(
                            out=recip_t[:, g, :], in_=psc[:, g, DK : DK + 1].bitcast(f32)
                        )
                        nc.vector.tensor_scalar_mul(
                            ctx_sb[:, qc * 4 + g, :],
                            psc[:, g, :DK].bitcast(f32),
                            recip_t[:, g, :],
                        )

                    # ---- stage C: attn tiles for this chunk ----
                    attn_t = attnp.tile([P, 4, S], f32, tag="attn")
                    for g in range(4):
                        gt = qc * 4 + g
                        # re-matmul path: columns [0, KS)
                        c = 0
                        while c < KS // CW:
                            take = 2 if c + 1 < KS // CW else 1
                            sps = mmps.tile([P, 2, CW], f32, tag="mm")
                            for j in range(take):
                                nc.tensor.matmul(
                                    sps[:, j, :],
                                    lhsT=qt[:, gt * P : (gt + 1) * P],
                                    rhs=kt_b[:, (c + j) * CW : (c + j + 1) * CW],
                                    start=True,
                                    stop=True,
                                )
                            nc.scalar.activation(
                                out=attn_t[:, g, c * CW : (c + take) * CW].rearrange(
                                    "q (j c) -> q j c", j=take
                                ),
                                in_=sps[:, :take, :],
                                func=AF.Exp,
                                scale=float(SCALE),
                            )
                            c += take
                        if KTS:
                            nc.gpsimd.tensor_scalar_mul(
                                attn_t[:, g, :KS], attn_t[:, g, :KS], recip_t[:, g, :]
                            )
                        # transpose path: columns [KS, S)
                        for kt4 in range(KTS, NT, 4):
                            tps = trps.tile([P, 4, P], f32r, tag="tr")
                            for j in range(4):
                                nc.tensor.transpose(
                                    tps[:, j, :],
                                    et[:, kt4 + j, g * P : (g + 1) * P],
                                    ident,
                                )
                            nc.vector.tensor_scalar_mul(
                                attn_t[:, g, kt4 * P : (kt4 + 4) * P].rearrange(
                                    "q (j c) -> q j c", j=4
                                ),
                                tps.bitcast(f32),
                                recip_t[:, g, :],
                            )
                    nc.sync.dma_start(
                        out=attn_out[pr].rearrange("(p t) k -> p t k", t=NT)[
                            :, qc * 4 : qc * 4 + 4, :
                        ],
                        in_=attn_t,
                    )

                nc.sync.dma_start(
                    out=ctx_out[pr].rearrange("(p t) d -> p t d", t=NT),
                    in_=ctx_sb,
                )

    nc.compile()
    return nc


_CACHE = {}


def _get_bass(**kw):
    key = tuple(sorted(kw.items()))
    if key not in _CACHE:
        _CACHE[key] = build_bass(**kw)
    return _CACHE[key]


def kernel(Q, K, V, attn_mask=None):
    """Full-input entry point: Q,K,V [4,8,2048,64] fp32 -> (context, attn)."""
    from concourse.bass_utils import run_bass_kernel_spmd

    Bq, Hq, S, D = Q.shape
    n_pairs = Bq * Hq
    QQ = np.ascontiguousarray(np.asarray(Q, dtype=np.float32).reshape(n_pairs, S, D))
    KK = np.ascontiguousarray(np.asarray(K, dtype=np.float32).reshape(n_pairs, S, D))
    VV = np.ascontiguousarray(np.asarray(V, dtype=np.float32).reshape(n_pairs, S, D))

    nc = _get_bass()
    per = n_pairs // N_CORES
    in_maps = [
        {
            "q_in": QQ[c * per : (c + 1) * per],
            "k_in": KK[c * per : (c + 1) * per],
            "v_in": VV[c * per : (c + 1) * per],
        }
        for c in range(N_CORES)
    ]
    res = run_bass_kernel_spmd(nc, in_maps, core_ids=list(range(N_CORES)))
    attn = np.concatenate([r["attn_out"] for r in res.results], axis=0).reshape(
        Bq, Hq, S, S
    )
    ctx = np.concatenate([r["ctx_out"] for r in res.results], axis=0).reshape(
        Bq, Hq, S, D
    )
    return ctx, attn
